# revision 10
# baseline (speedup 1.0000x reference)
"""Single-head causal attention on 8 TRN2 NeuronCores.

Problem: x[8, 2048, 1024] f32; Wq/Wk/Wv[1024, 128]; bq/bk/bv[128].
  q = x@Wq+bq; k = x@Wk+bk; v = x@Wv+bv
  scores[b,t,s] = k[b,t,:].q[b,s,:] / sqrt(128), causal (s<=t), softmax over s
  out = weights @ v   -> [8, 2048, 128] f32

Sharding: data-parallel over batch, one batch element per core. No collectives.

Per-core algorithm (T=2048, D=1024, H=128), matmuls in bf16:
  - host passes xT = x[b].T as bf16 [1024, 2048] (contraction dim on partitions)
    and W pre-chunked as [128, 8, 128].
  - qT/kT/vT [h, t] = W.T @ xT on PE, d-chunk outer so weights are reused and
    each chunk's matmuls start as soon as its xT DMA lands. Biases are applied
    per-partition in the PSUM->SBUF copy (DVE tensor_scalar_add, casts to bf16).
  - v is re-laid-out to [s, h] via 16 PE transposes; a ones column is appended
    so the P@V matmul also produces the softmax denominator.
  - scores are computed TRANSPOSED, row-major: S_T[s-tile, t] = qT.T @ kT so
    P_T = exp(S_T) is directly the stationary operand of out[t,129] = P_T.T @
    v_aug. No max-subtraction needed: scores are ~N(0, 0.33) by construction.
  - causal: blocks with si > tj are never computed; diagonal blocks get a 0/1
    multiplicative mask post-exp (DVE).
  - O phase, banded by 4 t-tiles: out[t,0:128]*reciprocal(out[t,128]) on DVE,
    then DMA out.
"""

import math

import ml_dtypes
import numpy as np

import concourse.bass as bass
import concourse.mybir as mybir
import concourse.tile as tile
from concourse import bacc
from concourse.bass_utils import run_bass_kernel_spmd

B, T, D, H = 8, 2048, 1024, 128
NT = T // 128          # 16 t/s tiles
NBAND = 4              # t-tiles per O band
ND = D // 128          # 8 contraction chunks
SCALE = 1.0 / math.sqrt(H)

F32 = mybir.dt.float32
BF16 = mybir.dt.bfloat16
AF = mybir.ActivationFunctionType


def build_nc():
    nc = bacc.Bacc(
        "TRN2",
        target_bir_lowering=False,
        debug=False,
        num_devices=8,
    )

    xT_d = nc.dram_tensor("xT", [D, T], BF16, kind="ExternalInput")
    w_d = {
        p: nc.dram_tensor(f"w{p}", [128, ND, H], BF16, kind="ExternalInput")
        for p in ("q", "k", "v")
    }
    bias_d = nc.dram_tensor("bias", [H, 3], F32, kind="ExternalInput")
    mask_d = nc.dram_tensor("mask", [128, 128], BF16, kind="ExternalInput")
    ident_d = nc.dram_tensor("ident", [128, 128], BF16, kind="ExternalInput")
    out_d = nc.dram_tensor("out", [T, H], F32, kind="ExternalOutput")

    with tile.TileContext(nc) as tc:
        with (
            tc.tile_pool(name="const", bufs=1) as const_pool,
            tc.tile_pool(name="x", bufs=1) as x_pool,
            tc.tile_pool(name="qkv", bufs=1) as qkv_pool,
            tc.tile_pool(name="vrows", bufs=1) as v_pool,
            tc.tile_pool(name="prows", bufs=1) as p_pool,
            tc.tile_pool(name="eps", bufs=3) as ep_pool,
        ):
            # ---- input DMAs, ordered so the first q matmul starts ASAP ----
            w_sb = {}
            for p in ("q", "k", "v"):
                w_sb[p] = const_pool.tile(
                    [128, ND, H], BF16, tag=f"w{p}", name=f"w{p}_sb"
                )
            bias_sb = const_pool.tile([128, 3], F32, tag="bias")
            mask_sb = const_pool.tile([128, 128], BF16, tag="mask")
            ident_sb = const_pool.tile([128, 128], BF16, tag="ident")
            xt = [
                x_pool.tile([128, T], BF16, tag=f"x{dc}", name=f"x{dc}_sb")
                for dc in range(ND)
            ]

            def dma_x(dc, piece):
                c0 = piece * 512
                nc.sync.dma_start(
                    xt[dc][:, c0 : c0 + 512],
                    xT_d[dc * 128 : (dc + 1) * 128, c0 : c0 + 512],
                )

            nc.sync.dma_start(w_sb["k"][:], w_d["k"][:])
            dma_x(0, 0)
            nc.sync.dma_start(w_sb["q"][:], w_d["q"][:])
            dma_x(0, 1)
            nc.sync.dma_start(w_sb["v"][:], w_d["v"][:])
            dma_x(0, 2)
            nc.sync.dma_start(bias_sb[:], bias_d[:])
            dma_x(0, 3)
            for dc in range(1, ND):
                for piece in range(4):
                    dma_x(dc, piece)
            nc.sync.dma_start(mask_sb[:], mask_d[:])
            nc.sync.dma_start(ident_sb[:], ident_d[:])
            # pre-warm the ACT exp table during the DMA wait
            warm = const_pool.tile([128, 1], F32, tag="warm")
            nc.scalar.activation(warm[:], bias_sb[:, 0:1], AF.Exp, scale=0.0)

            with tc.tile_pool(name="qkvps", bufs=2, space="PSUM") as qkv_ps:
                # ---- q,k projections: [h, t] bf16, bias folded in ----
                # d-chunk outer: matmuls start as each xT chunk DMA lands.
                # v is deferred into the band loop below so S/exp start earlier.
                proj_sb = {p: [None] * 4 for p in ("q", "k", "v")}
                PIDX = {"q": 0, "k": 1, "v": 2}

                def proj_copy(p, ps_t, ncol, col_off=None):
                    co = ncol * 512 if col_off is None else col_off
                    sb_t = qkv_pool.tile(
                        [128, 512], BF16, tag=f"{p}{ncol}", name=f"{p}T{ncol}_sb"
                    )
                    nc.vector.tensor_scalar_add(
                        sb_t[:],
                        ps_t[:, co : co + 512],
                        bias_sb[:, PIDX[p] : PIDX[p] + 1],
                    )
                    proj_sb[p][ncol] = sb_t

                for p in ("k", "q"):
                    ps_t = qkv_ps.tile([128, T], F32, name=f"ps_{p}", tag="qkvps")
                    for dc in range(ND):
                        for ncol in range(4):
                            nc.tensor.matmul(
                                ps_t[:, ncol * 512 : (ncol + 1) * 512],
                                w_sb[p][:, dc, :],
                                xt[dc][:, ncol * 512 : (ncol + 1) * 512],
                                start=(dc == 0),
                                stop=(dc == ND - 1),
                            )
                    for ncol in range(4):
                        proj_copy(p, ps_t, ncol)

            with (
                tc.tile_pool(name="sps", bufs=3, space="PSUM") as s_ps_pool,
                tc.tile_pool(name="ops", bufs=5, space="PSUM") as o_ps_pool,
            ):
                # ---- interleaved per band b: v chunk b -> S rows -> v
                # transposes -> O band. v's matmuls fill PE slack while ACT
                # chews exp; everything v is ready exactly when O needs it.
                v_rows = [None] * NT
                p_rows = []
                for b in range(NT // NBAND):
                    lo = b * NBAND
                    # v projection chunk b (rows 4b..4b+3 of v)
                    ps_v = s_ps_pool.tile([128, 512], F32, name=f"ps_v{b}", tag="sps")
                    for dc in range(ND):
                        nc.tensor.matmul(
                            ps_v[:],
                            w_sb["v"][:, dc, :],
                            xt[dc][:, lo * 128 : (lo + 4) * 128],
                            start=(dc == 0),
                            stop=(dc == ND - 1),
                        )
                    proj_copy("v", ps_v, b, col_off=0)
                    for si in range(lo, lo + NBAND):
                        gc0 = si * 128  # first valid global col (causal)
                        pr = p_pool.tile(
                            [128, T - gc0], BF16, tag=f"p{si}", name=f"p{si}_sb"
                        )
                        c = gc0
                        while c < T:
                            ce = min(T, (c // 512 + 1) * 512)
                            s_ps = s_ps_pool.tile(
                                [128, 512], F32, name=f"s_ps_{si}_{c}", tag="sps"
                            )
                            nc.tensor.matmul(
                                s_ps[:, 0 : ce - c],
                                proj_sb["q"][si // 4][
                                    :, (si % 4) * 128 : (si % 4 + 1) * 128
                                ],
                                proj_sb["k"][c // 512][:, c % 512 : c % 512 + (ce - c)],
                                start=True,
                                stop=True,
                            )
                            nc.scalar.activation(
                                pr[:, c - gc0 : ce - gc0],
                                s_ps[:, 0 : ce - c],
                                AF.Exp,
                                scale=SCALE,
                            )
                            c = ce
                        # diagonal block: causal mask (keep s <= t)
                        nc.vector.tensor_mul(pr[:, 0:128], pr[:, 0:128], mask_sb[:])
                        p_rows.append(pr)

                    for si in range(lo, lo + NBAND):
                        tp = o_ps_pool.tile(
                            [128, 129], BF16, name=f"vt_ps{si}", tag="ops"
                        )
                        nc.tensor.transpose(
                            tp[:, 0:128],
                            proj_sb["v"][b][:, (si - lo) * 128 : (si - lo + 1) * 128],
                            ident_sb[:],
                        )
                        vr = v_pool.tile(
                            [128, 129], BF16, tag=f"v{si}", name=f"v{si}_sb"
                        )
                        nc.vector.tensor_copy(vr[:, 0:128], tp[:, 0:128])
                        nc.vector.memset(vr[:, 128:129], 1.0)
                        v_rows[si] = vr

                    o_tiles = [
                        o_ps_pool.tile([128, 129], F32, name=f"o_ps_{b}_{j}", tag="ops")
                        for j in range(NBAND)
                    ]
                    for si in range(lo + NBAND):
                        for tj in range(max(si, lo), lo + NBAND):
                            nc.tensor.matmul(
                                o_tiles[tj - lo][:],
                                p_rows[si][:, (tj - si) * 128 : (tj - si + 1) * 128],
                                v_rows[si][:],
                                start=(si == 0),
                                stop=(si == tj),
                            )
                        if si >= lo:  # epilogue for t-tile tj == si
                            o_ps = o_tiles[si - lo]
                            recip = ep_pool.tile([128, 1], F32, tag="recip")
                            nc.vector.reciprocal(recip[:], o_ps[:, 128:129])
                            out_sb = ep_pool.tile([128, 128], F32, tag="outsb")
                            nc.vector.tensor_scalar_mul(
                                out_sb[:], o_ps[:, 0:128], recip[:, 0:1]
                            )
                            nc.sync.dma_start(
                                out_d[si * 128 : (si + 1) * 128, :], out_sb[:]
                            )

    nc.compile()
    return nc


_NC = None


def _get_nc():
    global _NC
    if _NC is None:
        _NC = build_nc()
    return _NC


def _make_in_maps(x, Wq, bq, Wk, bk, Wv, bv):
    bf = ml_dtypes.bfloat16

    def chunk_w(w):  # [1024, 128] -> [128, 8, 128] (partition, d-chunk, h)
        return np.ascontiguousarray(
            w.astype(bf).reshape(ND, 128, H).transpose(1, 0, 2)
        )

    shared = {
        "wq": chunk_w(Wq),
        "wk": chunk_w(Wk),
        "wv": chunk_w(Wv),
        "bias": np.ascontiguousarray(
            np.stack([bq, bk, bv], axis=1).astype(np.float32)
        ),
        "mask": np.triu(np.ones((128, 128), dtype=np.float32)).astype(bf),
        "ident": np.eye(128, dtype=np.float32).astype(bf),
    }
    in_maps = []
    for i in range(B):
        m = dict(shared)
        m["xT"] = np.ascontiguousarray(x[i].astype(bf).T)
        in_maps.append(m)
    return in_maps


def _run(inputs, trace=False, **kw):
    nc = _get_nc()
    in_maps = _make_in_maps(**inputs)
    res = run_bass_kernel_spmd(nc, in_maps, core_ids=list(range(B)), trace=trace, **kw)
    out = np.stack([res.results[i]["out"] for i in range(B)], axis=0)
    return out.astype(np.float32), res


def kernel(x, Wq, bq, Wk, bk, Wv, bv):
    out, _ = _run(dict(x=x, Wq=Wq, bq=bq, Wk=Wk, bk=bk, Wv=Wv, bv=bv))
    return out


# revision 11
# speedup vs baseline: 1.1809x; 1.1809x over previous
"""Single-head causal attention on 8 TRN2 NeuronCores.

Problem: x[8, 2048, 1024] f32; Wq/Wk/Wv[1024, 128]; bq/bk/bv[128].
  q = x@Wq+bq; k = x@Wk+bk; v = x@Wv+bv
  scores[b,t,s] = k[b,t,:].q[b,s,:] / sqrt(128), causal (s<=t), softmax over s
  out = weights @ v   -> [8, 2048, 128] f32

Sharding: data-parallel over batch, one batch element per core. No collectives.

Per-core algorithm (T=2048, D=1024, H=128), matmuls in bf16:
  - host passes xT = x[b].T as bf16 [1024, 2048] (contraction dim on partitions)
    and W pre-chunked as [128, 8, 128].
  - qT/kT/vT [h, t] = W.T @ xT on PE, d-chunk outer so weights are reused and
    each chunk's matmuls start as soon as its xT DMA lands. Biases are applied
    per-partition in the PSUM->SBUF copy (DVE tensor_scalar_add, casts to bf16).
  - v is re-laid-out to [s, h] via 16 PE transposes; a ones column is appended
    so the P@V matmul also produces the softmax denominator.
  - scores are computed TRANSPOSED, row-major: S_T[s-tile, t] = qT.T @ kT so
    P_T = exp(S_T) is directly the stationary operand of out[t,129] = P_T.T @
    v_aug. No max-subtraction needed: scores are ~N(0, 0.33) by construction.
  - causal: blocks with si > tj are never computed; diagonal blocks get a 0/1
    multiplicative mask post-exp (DVE).
  - O phase, banded by 4 t-tiles: out[t,0:128]*reciprocal(out[t,128]) on DVE,
    then DMA out.
"""

import math

import ml_dtypes
import numpy as np

import concourse.bass as bass
import concourse.mybir as mybir
import concourse.tile as tile
from concourse import bacc
from concourse.bass_utils import run_bass_kernel_spmd

B, T, D, H = 8, 2048, 1024, 128
NT = T // 128          # 16 t/s tiles
NBAND = 4              # t-tiles per O band
ND = D // 128          # 8 contraction chunks
SCALE = 1.0 / math.sqrt(H)

F32 = mybir.dt.float32
BF16 = mybir.dt.bfloat16
AF = mybir.ActivationFunctionType


def build_nc():
    nc = bacc.Bacc(
        "TRN2",
        target_bir_lowering=False,
        debug=False,
        num_devices=8,
    )

    xT_d = nc.dram_tensor("xT", [D, T], BF16, kind="ExternalInput")
    w_d = {
        p: nc.dram_tensor(f"w{p}", [128, ND, H], BF16, kind="ExternalInput")
        for p in ("q", "k", "v")
    }
    bias_d = nc.dram_tensor("bias", [H, 3], F32, kind="ExternalInput")
    mask_d = nc.dram_tensor("mask", [128, 128], BF16, kind="ExternalInput")
    ident_d = nc.dram_tensor("ident", [128, 128], BF16, kind="ExternalInput")
    out_d = nc.dram_tensor("out", [T, H], F32, kind="ExternalOutput")

    with tile.TileContext(nc) as tc:
        with (
            tc.tile_pool(name="const", bufs=1) as const_pool,
            tc.tile_pool(name="x", bufs=1) as x_pool,
            tc.tile_pool(name="qkv", bufs=1) as qkv_pool,
            tc.tile_pool(name="vrows", bufs=1) as v_pool,
            tc.tile_pool(name="prows", bufs=1) as p_pool,
            tc.tile_pool(name="eps", bufs=3) as ep_pool,
        ):
            # ---- input DMAs, ordered so the first q matmul starts ASAP ----
            w_sb = {}
            for p in ("q", "k", "v"):
                w_sb[p] = const_pool.tile(
                    [128, ND, H], BF16, tag=f"w{p}", name=f"w{p}_sb"
                )
            bias_sb = const_pool.tile([128, 3], F32, tag="bias")
            mask_sb = const_pool.tile([128, 128], BF16, tag="mask")
            ident_sb = const_pool.tile([128, 128], BF16, tag="ident")
            xt = [
                x_pool.tile([128, T], BF16, tag=f"x{dc}", name=f"x{dc}_sb")
                for dc in range(ND)
            ]

            nc.sync.dma_start(w_sb["q"][:], w_d["q"][:])
            for piece in range(4):  # x0 in pieces so the first matmul starts early
                nc.sync.dma_start(
                    xt[0][:, piece * 512 : (piece + 1) * 512],
                    xT_d[0:128, piece * 512 : (piece + 1) * 512],
                )
            nc.sync.dma_start(w_sb["v"][:], w_d["v"][:])
            nc.sync.dma_start(w_sb["k"][:], w_d["k"][:])
            nc.sync.dma_start(bias_sb[:], bias_d[:])
            for dc in range(1, ND):
                nc.sync.dma_start(xt[dc][:], xT_d[dc * 128 : (dc + 1) * 128, :])
            nc.sync.dma_start(mask_sb[:], mask_d[:])
            nc.sync.dma_start(ident_sb[:], ident_d[:])
            # pre-warm the ACT exp table during the DMA wait
            warm = const_pool.tile([128, 1], F32, tag="warm")
            nc.scalar.activation(warm[:], bias_sb[:, 0:1], AF.Exp, scale=0.0)

            with tc.tile_pool(name="qkvps", bufs=2, space="PSUM") as qkv_ps:
                # ---- projections: [h, t] bf16, bias folded in ----
                # q: d-chunk outer (matmuls start as each xT chunk DMA lands);
                # v, k: ncol outer so each 512-col chunk's PSUM->SBUF copy
                # pipelines under later matmuls. k last, with its copies split
                # ACT/DVE so the PSUM pool hands off to the band loop fast.
                proj_sb = {p: [None] * 4 for p in ("q", "k", "v")}
                PIDX = {"q": 0, "k": 1, "v": 2}

                def proj_copy(p, ps_t, ncol, split=False):
                    co = ncol * 512
                    sb_t = qkv_pool.tile(
                        [128, 512], BF16, tag=f"{p}{ncol}", name=f"{p}T{ncol}_sb"
                    )
                    bias_ap = bias_sb[:, PIDX[p] : PIDX[p] + 1]
                    if split:
                        nc.vector.tensor_scalar_add(
                            sb_t[:, 0:256], ps_t[:, co : co + 256], bias_ap
                        )
                        nc.scalar.activation(
                            sb_t[:, 256:512],
                            ps_t[:, co + 256 : co + 512],
                            AF.Identity,
                            bias=bias_ap,
                        )
                    else:
                        nc.vector.tensor_scalar_add(
                            sb_t[:], ps_t[:, co : co + 512], bias_ap
                        )
                    proj_sb[p][ncol] = sb_t

                ps_q = qkv_ps.tile([128, T], F32, name="ps_q", tag="qkvps")
                for dc in range(ND):
                    for ncol in range(4):
                        nc.tensor.matmul(
                            ps_q[:, ncol * 512 : (ncol + 1) * 512],
                            w_sb["q"][:, dc, :],
                            xt[dc][:, ncol * 512 : (ncol + 1) * 512],
                            start=(dc == 0),
                            stop=(dc == ND - 1),
                        )
                for ncol in range(4):
                    proj_copy("q", ps_q, ncol)

                for p in ("v", "k"):
                    ps_t = qkv_ps.tile([128, T], F32, name=f"ps_{p}", tag="qkvps")
                    for ncol in range(4):
                        for dc in range(ND):
                            nc.tensor.matmul(
                                ps_t[:, ncol * 512 : (ncol + 1) * 512],
                                w_sb[p][:, dc, :],
                                xt[dc][:, ncol * 512 : (ncol + 1) * 512],
                                start=(dc == 0),
                                stop=(dc == ND - 1),
                            )
                        proj_copy(p, ps_t, ncol, split=(p == "k"))

            with (
                tc.tile_pool(name="sps", bufs=3, space="PSUM") as s_ps_pool,
                tc.tile_pool(name="ops", bufs=5, space="PSUM") as o_ps_pool,
            ):
                # ---- interleaved per band b: v chunk b -> S rows -> v
                # transposes -> O band. v's matmuls fill PE slack while ACT
                # chews exp; everything v is ready exactly when O needs it.
                v_rows = [None] * NT
                p_rows = []
                for b in range(NT // NBAND):
                    lo = b * NBAND
                    for si in range(lo, lo + NBAND):
                        gc0 = si * 128  # first valid global col (causal)
                        pr = p_pool.tile(
                            [128, T - gc0], BF16, tag=f"p{si}", name=f"p{si}_sb"
                        )
                        c = gc0
                        while c < T:
                            ce = min(T, (c // 512 + 1) * 512)
                            s_ps = s_ps_pool.tile(
                                [128, 512], F32, name=f"s_ps_{si}_{c}", tag="sps"
                            )
                            nc.tensor.matmul(
                                s_ps[:, 0 : ce - c],
                                proj_sb["q"][si // 4][
                                    :, (si % 4) * 128 : (si % 4 + 1) * 128
                                ],
                                proj_sb["k"][c // 512][:, c % 512 : c % 512 + (ce - c)],
                                start=True,
                                stop=True,
                            )
                            nc.scalar.activation(
                                pr[:, c - gc0 : ce - gc0],
                                s_ps[:, 0 : ce - c],
                                AF.Exp,
                                scale=SCALE,
                            )
                            c = ce
                        # diagonal block: causal mask (keep s <= t)
                        nc.vector.tensor_mul(pr[:, 0:128], pr[:, 0:128], mask_sb[:])
                        p_rows.append(pr)

                    for si in range(lo, lo + NBAND):
                        tp = o_ps_pool.tile(
                            [128, 129], BF16, name=f"vt_ps{si}", tag="ops"
                        )
                        nc.tensor.transpose(
                            tp[:, 0:128],
                            proj_sb["v"][b][:, (si - lo) * 128 : (si - lo + 1) * 128],
                            ident_sb[:],
                        )
                        vr = v_pool.tile(
                            [128, 129], BF16, tag=f"v{si}", name=f"v{si}_sb"
                        )
                        nc.vector.tensor_copy(vr[:, 0:128], tp[:, 0:128])
                        nc.vector.memset(vr[:, 128:129], 1.0)
                        v_rows[si] = vr

                    o_tiles = [
                        o_ps_pool.tile([128, 129], F32, name=f"o_ps_{b}_{j}", tag="ops")
                        for j in range(NBAND)
                    ]
                    for si in range(lo + NBAND):
                        for tj in range(max(si, lo), lo + NBAND):
                            nc.tensor.matmul(
                                o_tiles[tj - lo][:],
                                p_rows[si][:, (tj - si) * 128 : (tj - si + 1) * 128],
                                v_rows[si][:],
                                start=(si == 0),
                                stop=(si == tj),
                            )
                        if si >= lo:  # epilogue for t-tile tj == si
                            o_ps = o_tiles[si - lo]
                            recip = ep_pool.tile([128, 1], F32, tag="recip")
                            nc.vector.reciprocal(recip[:], o_ps[:, 128:129])
                            out_sb = ep_pool.tile([128, 128], F32, tag="outsb")
                            nc.vector.tensor_scalar_mul(
                                out_sb[:], o_ps[:, 0:128], recip[:, 0:1]
                            )
                            nc.sync.dma_start(
                                out_d[si * 128 : (si + 1) * 128, :], out_sb[:]
                            )

    nc.compile()
    return nc


_NC = None


def _get_nc():
    global _NC
    if _NC is None:
        _NC = build_nc()
    return _NC


def _make_in_maps(x, Wq, bq, Wk, bk, Wv, bv):
    bf = ml_dtypes.bfloat16

    def chunk_w(w):  # [1024, 128] -> [128, 8, 128] (partition, d-chunk, h)
        return np.ascontiguousarray(
            w.astype(bf).reshape(ND, 128, H).transpose(1, 0, 2)
        )

    shared = {
        "wq": chunk_w(Wq),
        "wk": chunk_w(Wk),
        "wv": chunk_w(Wv),
        "bias": np.ascontiguousarray(
            np.stack([bq, bk, bv], axis=1).astype(np.float32)
        ),
        "mask": np.triu(np.ones((128, 128), dtype=np.float32)).astype(bf),
        "ident": np.eye(128, dtype=np.float32).astype(bf),
    }
    in_maps = []
    for i in range(B):
        m = dict(shared)
        m["xT"] = np.ascontiguousarray(x[i].astype(bf).T)
        in_maps.append(m)
    return in_maps


def _run(inputs, trace=False, **kw):
    nc = _get_nc()
    in_maps = _make_in_maps(**inputs)
    res = run_bass_kernel_spmd(nc, in_maps, core_ids=list(range(B)), trace=trace, **kw)
    out = np.stack([res.results[i]["out"] for i in range(B)], axis=0)
    return out.astype(np.float32), res


def kernel(x, Wq, bq, Wk, bk, Wv, bv):
    out, _ = _run(dict(x=x, Wq=Wq, bq=bq, Wk=Wk, bk=bk, Wv=Wv, bv=bv))
    return out


# revision 12
# speedup vs baseline: 1.3210x; 1.1186x over previous
"""Single-head causal attention on 8 TRN2 NeuronCores.

Problem: x[8, 2048, 1024] f32; Wq/Wk/Wv[1024, 128]; bq/bk/bv[128].
  q = x@Wq+bq; k = x@Wk+bk; v = x@Wv+bv
  scores[b,t,s] = k[b,t,:].q[b,s,:] / sqrt(128), causal (s<=t), softmax over s
  out = weights @ v   -> [8, 2048, 128] f32

Sharding: data-parallel over batch, one batch element per core. No collectives.

Per-core algorithm (T=2048, D=1024, H=128), matmuls in bf16:
  - host passes xT = x[b].T as bf16 [1024, 2048] (contraction dim on partitions)
    and W pre-chunked as [128, 8, 128].
  - qT/kT/vT [h, t] = W.T @ xT on PE, d-chunk outer so weights are reused and
    each chunk's matmuls start as soon as its xT DMA lands. Biases are applied
    per-partition in the PSUM->SBUF copy (DVE tensor_scalar_add, casts to bf16).
  - v is re-laid-out to [s, h] via 16 PE transposes; a ones column is appended
    so the P@V matmul also produces the softmax denominator.
  - scores are computed TRANSPOSED, row-major: S_T[s-tile, t] = qT.T @ kT so
    P_T = exp(S_T) is directly the stationary operand of out[t,129] = P_T.T @
    v_aug. No max-subtraction needed: scores are ~N(0, 0.33) by construction.
  - causal: blocks with si > tj are never computed; diagonal blocks get a 0/1
    multiplicative mask post-exp (DVE).
  - O phase, banded by 4 t-tiles: out[t,0:128]*reciprocal(out[t,128]) on DVE,
    then DMA out.
"""

import math

import ml_dtypes
import numpy as np

import concourse.bass as bass
import concourse.mybir as mybir
import concourse.tile as tile
from concourse import bacc
from concourse.bass_utils import run_bass_kernel_spmd

B, T, D, H = 8, 2048, 1024, 128
NT = T // 128          # 16 t/s tiles
NBAND = 4              # t-tiles per O band
ND = D // 128          # 8 contraction chunks
SCALE = 1.0 / math.sqrt(H)

F32 = mybir.dt.float32
BF16 = mybir.dt.bfloat16
AF = mybir.ActivationFunctionType


def build_nc():
    nc = bacc.Bacc(
        "TRN2",
        target_bir_lowering=False,
        debug=False,
        num_devices=8,
    )

    xT_d = nc.dram_tensor("xT", [D, T], BF16, kind="ExternalInput")
    w_d = {
        p: nc.dram_tensor(f"w{p}", [128, ND, H], BF16, kind="ExternalInput")
        for p in ("q", "k", "v")
    }
    bias_d = nc.dram_tensor("bias", [H, 3], F32, kind="ExternalInput")
    mask_d = nc.dram_tensor("mask", [128, 128], BF16, kind="ExternalInput")
    bvb_d = nc.dram_tensor("bvb", [128, 128], F32, kind="ExternalInput")
    out_d = nc.dram_tensor("out", [T, H], F32, kind="ExternalOutput")

    with tile.TileContext(nc) as tc:
        with (
            tc.tile_pool(name="const", bufs=1) as const_pool,
            tc.tile_pool(name="x", bufs=1) as x_pool,
            tc.tile_pool(name="qkv", bufs=1) as qkv_pool,
            tc.tile_pool(name="vrows", bufs=1) as v_pool,
            tc.tile_pool(name="prows", bufs=1) as p_pool,
            tc.tile_pool(name="eps", bufs=3) as ep_pool,
        ):
            # ---- input DMAs, ordered so the first q matmul starts ASAP ----
            w_sb = {}
            for p in ("q", "k", "v"):
                w_sb[p] = const_pool.tile(
                    [128, ND, H], BF16, tag=f"w{p}", name=f"w{p}_sb"
                )
            bias_sb = const_pool.tile([128, 3], F32, tag="bias")
            mask_sb = const_pool.tile([128, 128], BF16, tag="mask")
            bvb_sb = const_pool.tile([128, 128], F32, tag="bvb")
            xt = [
                x_pool.tile([128, T], BF16, tag=f"x{dc}", name=f"x{dc}_sb")
                for dc in range(ND)
            ]

            nc.sync.dma_start(w_sb["q"][:], w_d["q"][:])
            nc.sync.dma_start(w_sb["k"][:], w_d["k"][:])
            nc.sync.dma_start(xt[0][:], xT_d[0:128, :])
            nc.sync.dma_start(xt[1][:], xT_d[128:256, :])
            nc.sync.dma_start(w_sb["v"][:], w_d["v"][:])
            nc.sync.dma_start(bias_sb[:], bias_d[:])
            for dc in range(2, ND):
                nc.sync.dma_start(xt[dc][:], xT_d[dc * 128 : (dc + 1) * 128, :])
            nc.sync.dma_start(mask_sb[:], mask_d[:])
            nc.sync.dma_start(bvb_sb[:], bvb_d[:])
            # pre-warm the ACT exp table during the DMA wait
            warm = const_pool.tile([128, 1], F32, tag="warm")
            nc.scalar.activation(warm[:], bias_sb[:, 0:1], AF.Exp, scale=0.0)

            with tc.tile_pool(name="qkvps", bufs=2, space="PSUM") as qkv_ps:
                # ---- projections: [h, t] bf16, bias folded in ----
                # q: d-chunk outer (matmuls start as each xT chunk DMA lands);
                # v, k: ncol outer so each 512-col chunk's PSUM->SBUF copy
                # pipelines under later matmuls. k last, with its copies split
                # ACT/DVE so the PSUM pool hands off to the band loop fast.
                proj_sb = {p: [None] * 4 for p in ("q", "k", "v")}
                PIDX = {"q": 0, "k": 1, "v": 2}

                def proj_copy(p, ps_t, ncol, split=False):
                    co = ncol * 512
                    sb_t = qkv_pool.tile(
                        [128, 512], BF16, tag=f"{p}{ncol}", name=f"{p}T{ncol}_sb"
                    )
                    bias_ap = bias_sb[:, PIDX[p] : PIDX[p] + 1]
                    if split:
                        nc.vector.tensor_scalar_add(
                            sb_t[:, 0:256], ps_t[:, co : co + 256], bias_ap
                        )
                        nc.scalar.activation(
                            sb_t[:, 256:512],
                            ps_t[:, co + 256 : co + 512],
                            AF.Identity,
                            bias=bias_ap,
                        )
                    else:
                        nc.vector.tensor_scalar_add(
                            sb_t[:], ps_t[:, co : co + 512], bias_ap
                        )
                    proj_sb[p][ncol] = sb_t

                ps_q = qkv_ps.tile([128, T], F32, name="ps_q", tag="qkvps")
                ps_k = qkv_ps.tile([128, T], F32, name="ps_k", tag="qkvps")
                for dc in range(ND):
                    for p, ps_t in (("q", ps_q), ("k", ps_k)):
                        for ncol in range(4):
                            nc.tensor.matmul(
                                ps_t[:, ncol * 512 : (ncol + 1) * 512],
                                w_sb[p][:, dc, :],
                                xt[dc][:, ncol * 512 : (ncol + 1) * 512],
                                start=(dc == 0),
                                stop=(dc == ND - 1),
                            )
                for p, ps_t in (("q", ps_q), ("k", ps_k)):
                    for ncol in range(4):
                        proj_copy(p, ps_t, ncol, split=True)

            with (
                tc.tile_pool(name="sps", bufs=3, space="PSUM") as s_ps_pool,
                tc.tile_pool(name="ops", bufs=5, space="PSUM") as o_ps_pool,
            ):
                # ---- interleaved per band b: v chunk b -> S rows -> v
                # transposes -> O band. v's matmuls fill PE slack while ACT
                # chews exp; everything v is ready exactly when O needs it.
                v_rows = [None] * NT
                p_rows = []
                for b in range(NT // NBAND):
                    lo = b * NBAND
                    # v rows 4b..4b+3 directly in [s, h] layout (xT as weights)
                    for si in range(lo, lo + NBAND):
                        vp = o_ps_pool.tile(
                            [128, 129], F32, name=f"v_ps{si}", tag="ops"
                        )
                        for dc in range(ND):
                            nc.tensor.matmul(
                                vp[:, 0:128],
                                xt[dc][:, si * 128 : (si + 1) * 128],
                                w_sb["v"][:, dc, :],
                                start=(dc == 0),
                                stop=(dc == ND - 1),
                            )
                        vr = v_pool.tile(
                            [128, 129], BF16, tag=f"v{si}", name=f"v{si}_sb"
                        )
                        nc.vector.tensor_copy(vr[:, 0:128], vp[:, 0:128])
                        nc.vector.memset(vr[:, 128:129], 1.0)
                        v_rows[si] = vr

                    for si in range(lo, lo + NBAND):
                        gc0 = si * 128  # first valid global col (causal)
                        pr = p_pool.tile(
                            [128, T - gc0], BF16, tag=f"p{si}", name=f"p{si}_sb"
                        )
                        c = gc0
                        while c < T:
                            ce = min(T, (c // 512 + 1) * 512)
                            s_ps = s_ps_pool.tile(
                                [128, 512], F32, name=f"s_ps_{si}_{c}", tag="sps"
                            )
                            nc.tensor.matmul(
                                s_ps[:, 0 : ce - c],
                                proj_sb["q"][si // 4][
                                    :, (si % 4) * 128 : (si % 4 + 1) * 128
                                ],
                                proj_sb["k"][c // 512][:, c % 512 : c % 512 + (ce - c)],
                                start=True,
                                stop=True,
                            )
                            nc.scalar.activation(
                                pr[:, c - gc0 : ce - gc0],
                                s_ps[:, 0 : ce - c],
                                AF.Exp,
                                scale=SCALE,
                            )
                            c = ce
                        # diagonal block: causal mask (keep s <= t)
                        nc.vector.tensor_mul(pr[:, 0:128], pr[:, 0:128], mask_sb[:])
                        p_rows.append(pr)

                    o_tiles = [
                        o_ps_pool.tile([128, 129], F32, name=f"o_ps_{b}_{j}", tag="ops")
                        for j in range(NBAND)
                    ]
                    for si in range(lo + NBAND):
                        for tj in range(max(si, lo), lo + NBAND):
                            nc.tensor.matmul(
                                o_tiles[tj - lo][:],
                                p_rows[si][:, (tj - si) * 128 : (tj - si + 1) * 128],
                                v_rows[si][:],
                                start=(si == 0),
                                stop=(si == tj),
                            )
                        if si >= lo:  # epilogue for t-tile tj == si
                            o_ps = o_tiles[si - lo]
                            recip = ep_pool.tile([128, 1], F32, tag="recip")
                            nc.vector.reciprocal(recip[:], o_ps[:, 128:129])
                            sc_sb = ep_pool.tile([128, 128], F32, tag="scsb")
                            nc.vector.tensor_scalar_mul(
                                sc_sb[:], o_ps[:, 0:128], recip[:, 0:1]
                            )
                            out_sb = ep_pool.tile([128, 128], F32, tag="outsb")
                            nc.vector.tensor_add(out_sb[:], sc_sb[:], bvb_sb[:])
                            nc.sync.dma_start(
                                out_d[si * 128 : (si + 1) * 128, :], out_sb[:]
                            )

    nc.compile()
    return nc


_NC = None


def _get_nc():
    global _NC
    if _NC is None:
        _NC = build_nc()
    return _NC


def _make_in_maps(x, Wq, bq, Wk, bk, Wv, bv):
    bf = ml_dtypes.bfloat16

    def chunk_w(w):  # [1024, 128] -> [128, 8, 128] (partition, d-chunk, h)
        return np.ascontiguousarray(
            w.astype(bf).reshape(ND, 128, H).transpose(1, 0, 2)
        )

    shared = {
        "wq": chunk_w(Wq),
        "wk": chunk_w(Wk),
        "wv": chunk_w(Wv),
        "bias": np.ascontiguousarray(
            np.stack([bq, bk, bv], axis=1).astype(np.float32)
        ),
        "mask": np.triu(np.ones((128, 128), dtype=np.float32)).astype(bf),
        "bvb": np.ascontiguousarray(
            np.broadcast_to(bv.astype(np.float32), (128, 128))
        ),
    }
    in_maps = []
    for i in range(B):
        m = dict(shared)
        m["xT"] = np.ascontiguousarray(x[i].astype(bf).T)
        in_maps.append(m)
    return in_maps


def _run(inputs, trace=False, **kw):
    nc = _get_nc()
    in_maps = _make_in_maps(**inputs)
    res = run_bass_kernel_spmd(nc, in_maps, core_ids=list(range(B)), trace=trace, **kw)
    out = np.stack([res.results[i]["out"] for i in range(B)], axis=0)
    return out.astype(np.float32), res


def kernel(x, Wq, bq, Wk, bk, Wv, bv):
    out, _ = _run(dict(x=x, Wq=Wq, bq=bq, Wk=Wk, bk=bk, Wv=Wv, bv=bv))
    return out
